# revision 55
# baseline (speedup 1.0000x reference)
"""Bass/Tile Trainium2 kernel for nn_BcosGCNLayer (b-cos linear layer, B=2).

reference:
    lin  = z @ W.T
    cos  = normalize(z) @ normalize(W).T
    out  = lin * |cos|**(B-1) = lin * |cos|          (B = 2)

Key identity: with W~ = W * ||w_row||^(-1/2) (row-wise) and
P = z @ W~.T, we get  P * |P| / ||z_n|| = lin * |cos| = out.
One GEMM; the epilogue is A = |P| * inv_zn (one ACT op — inv_zn rides the
activation's per-partition scale operand) then out = P * A (one DVE op).

The device program is **bf16-I/O** (build_kernel_bf16, the production
path): kernel() casts z to bf16 on the host, the NEFF reads/writes bf16
DRAM, and the host converts the bf16 out back to f32. HW probes showed
the f32 version was HBM-bound — its load+store stream ALONE took 183us of
the 195us pass (~280 GB/s/core mixed r+w); bf16 halves that traffic.
bf16 also makes the PE transposes 1 cycle/row and the GEMM pure-bf16
(f32 accumulate in PSUM). Measured global rel err ~6e-3 vs the 2e-2 gate.

Sharding: data-parallel on rows across 8 cores (12500 rows/core, padded
to 12544 = 98*128); weight replicated, prepped once per core (fused
scale+transpose matmuls into bf16 W~T tiles).

Layout: rows are processed in groups of 8 tiles (1024 rows) with the
row->partition mapping n = 8p + q, so one 1MB bf16 load/store DMA moves
8KB contiguous per partition. Per tile: ACT Square+accum (row norms),
4 PE transposes (bf16, identity moving operand), DVE copyback
(PSUM->SBUF), 4 bf16 GEMMs (ztile stationary, W~T 512-col moving), ACT
Abs-with-scale, DVE multiply, SWDGE store. Loads ride the HWDGE (sync)
queue, stores the SWDGE (gpsimd) queue so a store waiting on compute
never blocks a load.

Toolchain notes (this neuronxcc build): DVE tensor_scalar(abs_max,*),
scalar_tensor_tensor, tensor_tensor(abs_max) and tensor_tensor_reduce
all fail the backend ISA check — |P| must come from ACT Abs. GPSIMD
elementwise is ~15x slower than the cost model on HW (6.5us per
[128,512] op) — never offload per-tile math to Pool. Transpose-mode
matmul requires a true permutation as the moving operand (no diag-scale
fusion).
"""

import numpy as np

import concourse.bacc as bacc
import concourse.bass as bass
import concourse.mybir as mybir
import concourse.tile as tile
from concourse import masks

P = 128
D = 512
KB = D // P  # 4 blocks of 128 along the feature dim
GQ = 4  # rows per partition per group (group = GQ*P = 512 rows)
N_CORES = 8
TOTAL_ROWS = 100000
ROWS_PER_CORE_RAW = TOTAL_ROWS // N_CORES  # 12500
TILES_PER_CORE = -(-ROWS_PER_CORE_RAW // P)  # 98
ROWS_PER_CORE = TILES_PER_CORE * P  # 12544

F32 = mybir.dt.float32
F32R = mybir.dt.float32r
ACT = mybir.ActivationFunctionType

STORE_ENGINE = "gpsimd"
ABS_ON_DVE_EVERY = 0  # every Nth q-slice's abs runs on DVE instead of ACT (0=off)
NORMS_ENGINE = "act"  # "act": ACT Square+accum; "dve": DVE tensor_tensor_reduce
COPYBACK = "split"  # "split": odd q on ACT / even on DVE; "dve"; "act"
TP_IDENT = "f32"  # identity dtype for z transposes: "f32" | "f32r" | "bf16"
# (identity is the PE moving operand in transpose mode; its dtype sets the
# streaming rate: f32=2 cycles/row, f32r=1.5, bf16=1.0. x*1.0 is exact.)


def build_kernel(
    rows: int = ROWS_PER_CORE,
    repeat: int = 1,
    alias_rows: int = 0,
    hw_loop: int = 0,
    norms_engine: str | None = None,
    copyback: str | None = None,
    tp_ident: str | None = None,
    parts: str = "all",  # "all" | "load" | "dma" | "compute" (HW probes)
    **probe_kw,
) -> bass.Bass:
    """Build the per-core Bass program: z [rows, 512] -> out [rows, 512].

    repeat / alias_rows / hw_loop are bench-only knobs: alias_rows shrinks
    the DRAM tensors (addressing wraps) so host<->device shipping is tiny,
    hw_loop wraps the whole pass in a For_i, repeat emits several passes
    per loop iteration.
    """
    assert rows % P == 0
    if parts != "all":
        return build_probe_kernel(rows, repeat, alias_rows, hw_loop, parts, **probe_kw)
    assert not probe_kw, f"unknown kwargs {probe_kw}"
    norms_engine = norms_engine or NORMS_ENGINE
    copyback = copyback or COPYBACK
    tp_ident = tp_ident or TP_IDENT
    n_tiles = rows // P
    dram_rows = alias_rows or rows

    # groups of (tile0, qn): qn*P rows with row mapping n = tile0*P + qn*p + q
    groups = []
    r = 0
    while r < n_tiles:
        qn = min(GQ, n_tiles - r)
        groups.append((r, qn))
        r += qn

    nc = bacc.Bacc()
    z_dram = nc.dram_tensor("z", [dram_rows, D], F32, kind="ExternalInput")
    w_dram = nc.dram_tensor("w", [D, D], F32, kind="ExternalInput")
    out_dram = nc.dram_tensor("out", [dram_rows, D], F32, kind="ExternalOutput")

    def rowslice(dram, t0, qn):
        r0 = (t0 * P) % dram_rows
        return dram[r0 : r0 + qn * P, :].rearrange("(p q) d -> p (q d)", p=P, q=qn)

    with tile.TileContext(nc) as tc:
        with (
            tc.tile_pool(name="consts", bufs=1) as consts,
            tc.tile_pool(name="wprep", bufs=1) as wprep,
            tc.tile_pool(name="zin", bufs=8) as zin_pool,
            tc.tile_pool(name="scratch", bufs=1) as scratch_pool,
            tc.tile_pool(name="stats", bufs=8) as stats_pool,
            tc.tile_pool(name="zt", bufs=14) as zt_pool,
            tc.tile_pool(name="absb", bufs=6) as abs_pool,
            tc.tile_pool(name="outb", bufs=3) as out_pool,
            tc.tile_pool(name="psum_t", bufs=3, space=bass.MemorySpace.PSUM) as pt_pool,
            tc.tile_pool(name="psum_o", bufs=5, space=bass.MemorySpace.PSUM) as po_pool,
        ):
            ident = consts.tile([P, P], F32)
            masks.make_identity(nc, ident[:])
            if tp_ident == "bf16":
                ident_t = consts.tile([P, P], mybir.dt.bfloat16)
                nc.vector.tensor_copy(ident_t[:], ident[:])  # 1.0 exact in bf16
                tp_dt = F32R
            elif tp_ident == "f32r":
                ident_t = None  # bitcast view of ident at use site
                tp_dt = F32R
            else:
                ident_t = None
                tp_dt = F32
            # PE warmup: absorbs the identity-producer wait into a single
            # instruction so later PE ops carry at most one foreign wait
            # (TPB instructions have exactly one inline sem-wait slot).
            warm = pt_pool.tile([P, P], F32, name="psum_t")
            nc.tensor.transpose(warm[:], ident[:], ident[:])

            # persistent W~T tiles: [i-block k][i=128, o=512]
            wT = wprep.tile([P, KB, D], F32R)

            def batch_front(g):
                """One group: 1MB contiguous load, then per q-slice:
                Square-accum (ACT), 4 PE transposes, DVE copyback."""
                t0, qn = g
                zbig = zin_pool.tile([P, GQ, D], F32, name="z_nat")
                nc.sync.dma_start(
                    zbig[:, :qn, :].rearrange("p a b -> p (a b)"),
                    rowslice(z_dram, t0, qn),
                )
                ssq = stats_pool.tile([P, GQ], F32, name="ssq")
                ztiles = []
                for q in range(qn):
                    zq = zbig[:, q, :]
                    zsq_scr = scratch_pool.tile([P, D], F32, name="zsq_scr")
                    if norms_engine == "dve":
                        nc.vector.tensor_tensor_reduce(
                            zsq_scr[:], zq, zq, 1.0, 0.0,
                            mybir.AluOpType.mult, mybir.AluOpType.add,
                            accum_out=ssq[:, q : q + 1],
                        )
                    else:
                        nc.scalar.activation(
                            zsq_scr[:], zq, ACT.Square, accum_out=ssq[:, q : q + 1]
                        )
                    ptz = pt_pool.tile([P, KB, P], tp_dt, name="psum_t")
                    for k in range(KB):
                        zqk = zq[:, k * P : (k + 1) * P]
                        if tp_ident == "bf16":
                            nc.tensor.transpose(
                                ptz[:, k, :], zqk.bitcast(F32R), ident_t[:]
                            )
                        elif tp_ident == "f32r":
                            nc.tensor.transpose(
                                ptz[:, k, :], zqk.bitcast(F32R), ident[:].bitcast(F32R)
                            )
                        else:
                            nc.tensor.transpose(ptz[:, k, :], zqk, ident[:])
                    ztile = zt_pool.tile([P, KB, P], F32R, name="ztile")
                    on_act = copyback == "act" or (copyback == "split" and q % 2)
                    if on_act:
                        # ACT Copy needs no activation table → no switch penalty
                        nc.scalar.copy(
                            ztile[:].rearrange("p a b -> p (a b)"),
                            ptz[:].rearrange("p a b -> p (a b)"),
                        )
                    else:
                        nc.vector.tensor_copy(
                            ztile[:].rearrange("p a b -> p (a b)"),
                            ptz[:].rearrange("p a b -> p (a b)"),
                        )
                    ztiles.append(ztile)
                return ssq, ztiles

            def batch_back(g, ssq, ztiles):
                """GEMMs + inv-norm + epilogue + one 1MB store."""
                t0, qn = g
                pos = []
                for q in range(qn):
                    po = po_pool.tile([P, D], F32, name="psum_o")
                    for k in range(KB):
                        nc.tensor.matmul(
                            po[:],
                            ztiles[q][:, k, :],
                            wT[:, k, :],
                            start=(k == 0),
                            stop=(k == KB - 1),
                        )
                    pos.append(po)
                # inv_zn = sqrt(1/ssq): DVE reciprocal first so the final
                # ACT op (Sqrt) is the producer -> abs's scale dep stays
                # same-engine and the ACT stream is [Sq xqn][Sqrt][Abs xqn]
                # (every activation-table switch costs ~1us).
                zrec = stats_pool.tile([P, GQ], F32, name="zrec")
                nc.vector.reciprocal(zrec[:, :qn], ssq[:, :qn])
                zscale = stats_pool.tile([P, GQ], F32, name="zscale")
                nc.scalar.activation(zscale[:, :qn], zrec[:, :qn], ACT.Sqrt)
                og = out_pool.tile([P, GQ, D], F32, name="ot")
                for q in range(qn):
                    po = pos[q]
                    ab = abs_pool.tile([P, D], F32, name="ab")
                    t = t0 + q
                    if ABS_ON_DVE_EVERY and t % ABS_ON_DVE_EVERY == ABS_ON_DVE_EVERY - 1:
                        nc.vector.tensor_scalar(
                            ab[:], po[:], 0.0, zscale[:, q : q + 1],
                            mybir.AluOpType.abs_max, mybir.AluOpType.mult,
                        )
                    else:
                        nc.scalar.activation(
                            ab[:], po[:], ACT.Abs, scale=zscale[:, q : q + 1]
                        )
                    nc.vector.tensor_mul(og[:, q, :], po[:], ab[:])
                getattr(nc, STORE_ENGINE).dma_start(
                    rowslice(out_dram, t0, qn),
                    og[:, :qn, :].rearrange("p a b -> p (a b)"),
                )

            def w_prep_stats():
                """W load + norm-scale chain (no PE work): runs while the
                first z groups stream in."""
                w_nat = wprep.tile([P, KB, D], F32)
                nc.sync.dma_start(
                    w_nat[:], w_dram[:].rearrange("(b p) d -> p b d", p=P)
                )
                wsq_scratch = wprep.tile([P, D], F32)
                wssq = wprep.tile([P, KB], F32)
                for b in range(KB):
                    nc.scalar.activation(
                        wsq_scratch[:], w_nat[:, b, :], ACT.Square,
                        accum_out=wssq[:, b : b + 1],
                    )
                wnrm = wprep.tile([P, KB], F32)
                nc.scalar.activation(wnrm[:], wssq[:], ACT.Sqrt)  # ||w||
                wnrm2 = wprep.tile([P, KB], F32)
                nc.scalar.activation(wnrm2[:], wnrm[:], ACT.Sqrt)  # ||w||^(1/2)
                wscale = wprep.tile([P, KB], F32)
                nc.vector.reciprocal(wscale[:], wnrm2[:])  # ||w||^(-1/2)
                # DVE-sourced copies of both W-matmul operands so the W PE
                # matmuls wait on a single engine's semaphore.
                w_nat2 = wprep.tile([P, KB, D], F32)
                nc.vector.tensor_copy(
                    w_nat2[:].rearrange("p a b -> p (a b)"),
                    w_nat[:].rearrange("p a b -> p (a b)"),
                )
                # diag(s_w) per o-block, for the fused scale+transpose matmul
                dsw = wprep.tile([P, KB, P], F32)
                for b in range(KB):
                    nc.vector.tensor_scalar_mul(
                        dsw[:, b, :], ident[:], wscale[:, b : b + 1]
                    )
                return w_nat2, dsw

            def w_prep_pe(w_nat2, dsw):
                """One fused scale+transpose matmul per (o-block, i-block):
                W.T @ diag(s_w) = (s_w * W).T"""
                for k in range(KB):
                    pw = pt_pool.tile([P, KB, P], F32, name="psum_t")
                    for b in range(KB):
                        nc.tensor.matmul(
                            pw[:, b, :],
                            w_nat2[:, b, k * P : (k + 1) * P],
                            dsw[:, b, :],
                        )
                    nc.vector.tensor_copy(
                        wT[:, k, :], pw[:].rearrange("p a b -> p (a b)")
                    )

            LOOKAHEAD = 3

            def emit_passes(n_passes):
                all_groups = groups * n_passes
                fronts = {}
                for i in range(min(LOOKAHEAD, len(all_groups))):
                    fronts[i] = batch_front(all_groups[i])
                yield  # caller interleaves W-prep PE work here
                for i in range(len(all_groups)):
                    ssq, ztiles = fronts.pop(i)
                    batch_back(all_groups[i], ssq, ztiles)
                    if i + LOOKAHEAD < len(all_groups):
                        fronts[i + LOOKAHEAD] = batch_front(all_groups[i + LOOKAHEAD])

            w_nat2, dsw = w_prep_stats()
            if hw_loop:
                w_prep_pe(w_nat2, dsw)
                with tc.For_i(
                    0, hw_loop, 1,
                    hint_engines=(mybir.EngineType.PE, mybir.EngineType.Activation,
                                  mybir.EngineType.DVE, mybir.EngineType.SP,
                                  mybir.EngineType.Pool),
                ):
                    for _ in emit_passes(repeat):
                        pass
            else:
                gen = emit_passes(repeat)
                next(gen)
                w_prep_pe(w_nat2, dsw)
                for _ in gen:
                    pass

    nc.compile()
    return nc


def build_probe_kernel(
    rows: int,
    repeat: int = 1,
    alias_rows: int = 0,
    hw_loop: int = 0,
    parts: str = "dma",
    store_q: str = STORE_ENGINE,
    load_q: str = "sync",
    gq: int = GQ,
) -> bass.Bass:
    """Stripped kernels for HW bottleneck probing.

    parts="load": only the z load stream.  "dma": loads + stores (stores
    read a constant SBUF tile).  "compute": full compute pipeline reading
    one resident z group, no per-group DMA.
    """
    n_tiles = rows // P
    dram_rows = alias_rows or rows
    groups = []
    r = 0
    while r < n_tiles:
        qn = min(gq, n_tiles - r)
        groups.append((r, qn))
        r += qn

    nc = bacc.Bacc()
    z_dram = nc.dram_tensor("z", [dram_rows, D], F32, kind="ExternalInput")
    nc.dram_tensor("w", [D, D], F32, kind="ExternalInput")
    out_dram = nc.dram_tensor("out", [dram_rows, D], F32, kind="ExternalOutput")

    def rowslice(dram, t0, qn):
        r0 = (t0 * P) % dram_rows
        return dram[r0 : r0 + qn * P, :].rearrange("(p q) d -> p (q d)", p=P, q=qn)

    with tile.TileContext(nc) as tc:
        with (
            tc.tile_pool(name="consts", bufs=1) as consts,
            tc.tile_pool(name="zin", bufs=8) as zin_pool,
            tc.tile_pool(name="stats", bufs=8) as stats_pool,
            tc.tile_pool(name="zt", bufs=14) as zt_pool,
            tc.tile_pool(name="absb", bufs=6) as abs_pool,
            tc.tile_pool(name="outb", bufs=3) as out_pool,
            tc.tile_pool(name="psum_t", bufs=3, space=bass.MemorySpace.PSUM) as pt_pool,
            tc.tile_pool(name="psum_o", bufs=5, space=bass.MemorySpace.PSUM) as po_pool,
        ):
            if parts == "compute":
                ident = consts.tile([P, P], F32)
                masks.make_identity(nc, ident[:])
                warm = pt_pool.tile([P, P], F32, name="psum_t")
                nc.tensor.transpose(warm[:], ident[:], ident[:])
                wT = consts.tile([P, KB, D], F32R)
                nc.vector.memset(wT[:].rearrange("p a b -> p (a b)"), 0.001)
                zbig_c = consts.tile([P, gq, D], F32)
                nc.sync.dma_start(
                    zbig_c[:].rearrange("p a b -> p (a b)"), rowslice(z_dram, 0, gq)
                )
                scratch = consts.tile([P, D], F32)

                def front(g):
                    t0, qn = g
                    ssq = stats_pool.tile([P, gq], F32, name="ssq")
                    ztiles = []
                    for q in range(qn):
                        zq = zbig_c[:, q, :]
                        nc.scalar.activation(
                            scratch[:], zq, ACT.Square, accum_out=ssq[:, q : q + 1]
                        )
                        ptz = pt_pool.tile([P, KB, P], F32, name="psum_t")
                        for k in range(KB):
                            nc.tensor.transpose(
                                ptz[:, k, :], zq[:, k * P : (k + 1) * P], ident[:]
                            )
                        ztile = zt_pool.tile([P, KB, P], F32R, name="ztile")
                        if q % 2:
                            nc.scalar.copy(
                                ztile[:].rearrange("p a b -> p (a b)"),
                                ptz[:].rearrange("p a b -> p (a b)"),
                            )
                        else:
                            nc.vector.tensor_copy(
                                ztile[:].rearrange("p a b -> p (a b)"),
                                ptz[:].rearrange("p a b -> p (a b)"),
                            )
                        ztiles.append(ztile)
                    return ssq, ztiles

                def back(g, ssq, ztiles):
                    t0, qn = g
                    pos = []
                    for q in range(qn):
                        po = po_pool.tile([P, D], F32, name="psum_o")
                        for k in range(KB):
                            nc.tensor.matmul(
                                po[:], ztiles[q][:, k, :], wT[:, k, :],
                                start=(k == 0), stop=(k == KB - 1),
                            )
                        pos.append(po)
                    zrec = stats_pool.tile([P, GQ], F32, name="zrec")
                    nc.vector.reciprocal(zrec[:, :qn], ssq[:, :qn])
                    zscale = stats_pool.tile([P, GQ], F32, name="zscale")
                    nc.scalar.activation(zscale[:, :qn], zrec[:, :qn], ACT.Sqrt)
                    og = out_pool.tile([P, GQ, D], F32, name="ot")
                    for q in range(qn):
                        po = pos[q]
                        ab = abs_pool.tile([P, D], F32, name="ab")
                        nc.scalar.activation(
                            ab[:], po[:], ACT.Abs, scale=zscale[:, q : q + 1]
                        )
                        nc.vector.tensor_mul(og[:, q, :], po[:], ab[:])

            else:
                og_const = consts.tile([P, gq, D], F32)
                nc.vector.memset(og_const[:].rearrange("p a b -> p (a b)"), 0.0)

                def front(g):
                    t0, qn = g
                    zbig = zin_pool.tile([P, gq, D], F32, name="z_nat")
                    getattr(nc, load_q).dma_start(
                        zbig[:, :qn, :].rearrange("p a b -> p (a b)"),
                        rowslice(z_dram, t0, qn),
                    )
                    return None, None

                def back(g, ssq, ztiles):
                    if parts == "dma":
                        t0, qn = g
                        getattr(nc, store_q).dma_start(
                            rowslice(out_dram, t0, qn),
                            og_const[:, :qn, :].rearrange("p a b -> p (a b)"),
                        )

            LOOKAHEAD = 3

            def emit_passes(n_passes):
                all_groups = groups * n_passes
                fronts = {}
                for i in range(min(LOOKAHEAD, len(all_groups))):
                    fronts[i] = front(all_groups[i])
                for i in range(len(all_groups)):
                    ssq, ztiles = fronts.pop(i)
                    back(all_groups[i], ssq, ztiles)
                    if i + LOOKAHEAD < len(all_groups):
                        fronts[i + LOOKAHEAD] = front(all_groups[i + LOOKAHEAD])

            if hw_loop:
                with tc.For_i(
                    0, hw_loop, 1,
                    hint_engines=(mybir.EngineType.PE, mybir.EngineType.Activation,
                                  mybir.EngineType.DVE, mybir.EngineType.SP,
                                  mybir.EngineType.Pool),
                ):
                    emit_passes(repeat)
            else:
                emit_passes(repeat)

    nc.compile()
    return nc


BF16 = mybir.dt.bfloat16
GQB = 8  # tiles per group in the bf16 kernel (8KB/partition per 1MB DMA)


def build_kernel_bf16(
    rows: int = ROWS_PER_CORE,
    repeat: int = 1,
    alias_rows: int = 0,
    hw_loop: int = 0,
    scale_stage: str = "abs",  # "dve": batch-Abs + per-tile DVE og-scale;
    # "abs": per-tile Abs-with-scale
    fuse: int = 1,  # tiles per PSUM-out tile / epilogue op batch (1 or 2)
    pt_bufs: int = 2,  # PSUM transpose bufs; GEMM-out gets the rest
    zin_bufs: int = 6,
    zt_bufs: int = 28,
    emit_order: str = "back_first",  # "front_first": emit front(i+L) before
    # back(i) so every engine's FIFO sees ready work (Sq/T/copy) ahead of
    # dependency-waiting work (Abs/mul/GEMM) — avoids head-of-line stalls
    copyback: str = "dve",  # "dve" | "act" | "split"
    store_q: str = "gpsimd",
    load_q: str = "sync",
    gq: int = GQB,
    lookahead: int = 3,
    probe: str = "",  # "" | "dma" | "compute"
) -> bass.Bass:
    """bf16-I/O kernel: z [rows,512] bf16 -> out [rows,512] bf16.

    Same math as the f32 kernel (P = z @ W~.T; out = P*|P|*inv_zn with the
    inv_zn riding the Abs's scale operand), but all DRAM I/O is bf16 —
    halving HBM traffic, which the f32 probes showed is the binding
    constraint (load+store alone = 183us of the 195us pass). Transposes
    and GEMMs run pure bf16 (1 cycle/row); the epilogue Abs writes bf16;
    the out store is bf16. scalar_tensor_tensor / tensor_scalar /
    tensor_tensor(abs_max) / tensor_tensor_reduce all fail neuronxcc's
    ISA check on this build, so the epilogue stays ACT Abs + DVE mult.
    """
    assert rows % P == 0
    n_tiles = rows // P
    dram_rows = alias_rows or rows

    groups = []
    r = 0
    while r < n_tiles:
        qn = min(gq, n_tiles - r)
        groups.append((r, qn))
        r += qn

    nc = bacc.Bacc()
    z_dram = nc.dram_tensor("z", [dram_rows, D], BF16, kind="ExternalInput")
    w_dram = nc.dram_tensor("w", [D, D], F32, kind="ExternalInput")
    out_dram = nc.dram_tensor("out", [dram_rows, D], BF16, kind="ExternalOutput")

    def rowslice(dram, t0, qn):
        r0 = (t0 * P) % dram_rows
        return dram[r0 : r0 + qn * P, :].rearrange("(p q) d -> p (q d)", p=P, q=qn)

    with tile.TileContext(nc) as tc:
        with (
            tc.tile_pool(name="consts", bufs=1) as consts,
            tc.tile_pool(name="wprep", bufs=1) as wprep,
            tc.tile_pool(name="zin", bufs=zin_bufs) as zin_pool,
            tc.tile_pool(name="scratch", bufs=1) as scratch_pool,
            tc.tile_pool(name="stats", bufs=8) as stats_pool,
            tc.tile_pool(name="zt", bufs=zt_bufs) as zt_pool,
            tc.tile_pool(name="absb", bufs=4) as abs_pool,
            tc.tile_pool(name="outb", bufs=3) as out_pool,
            tc.tile_pool(
                name="psum_t", bufs=pt_bufs, space=bass.MemorySpace.PSUM
            ) as pt_pool,
            tc.tile_pool(
                name="psum_o",
                bufs=(16 - 2 * pt_bufs) // (2 * fuse),
                space=bass.MemorySpace.PSUM,
            ) as po_pool,
        ):
            ident = consts.tile([P, P], F32)
            masks.make_identity(nc, ident[:])
            ident_bf = consts.tile([P, P], BF16)
            nc.vector.tensor_copy(ident_bf[:], ident[:])  # 1.0 exact in bf16
            warm = pt_pool.tile([P, P], F32, name="psum_t")
            nc.tensor.transpose(warm[:], ident[:], ident[:])

            # persistent W~T tiles (bf16): [i-block k][i=128, o=512]
            wT = wprep.tile([P, KB, D], BF16)

            zbig_c = None
            og_c = None
            if probe == "compute":
                zbig_c = consts.tile([P, gq, D], BF16)
                nc.sync.dma_start(
                    zbig_c[:].rearrange("p a b -> p (a b)"), rowslice(z_dram, 0, gq)
                )
            if probe == "dma":
                og_c = consts.tile([P, gq, D], BF16)
                nc.vector.memset(og_c[:].rearrange("p a b -> p (a b)"), 0.0)

            def batch_front(g):
                t0, qn = g
                if probe == "compute":
                    zbig = zbig_c
                else:
                    zbig = zin_pool.tile([P, gq, D], BF16, name="z_nat")
                    getattr(nc, load_q).dma_start(
                        zbig[:, :qn, :].rearrange("p a b -> p (a b)"),
                        rowslice(z_dram, t0, qn),
                    )
                if probe == "dma":
                    return None, None
                # row sums of squares (bf16 in, f32 accum)
                ssq = stats_pool.tile([P, gq], F32, name="ssq")
                if probe != "pe":
                    for q in range(qn):
                        scr = scratch_pool.tile([P, D], BF16, name="zsq_scr")
                        nc.scalar.activation(
                            scr[:], zbig[:, q, :], ACT.Square,
                            accum_out=ssq[:, q : q + 1],
                        )
                ztiles = []
                for q in range(qn):
                    zq = zbig[:, q, :]
                    ptz = pt_pool.tile([P, KB, P], BF16, name="psum_t")
                    for k in range(KB):
                        nc.tensor.transpose(
                            ptz[:, k, :], zq[:, k * P : (k + 1) * P], ident_bf[:]
                        )
                    ztile = zt_pool.tile([P, KB, P], BF16, name="ztile")
                    on_act = copyback == "act" or (copyback == "split" and q % 2)
                    if on_act:
                        nc.scalar.copy(
                            ztile[:].rearrange("p a b -> p (a b)"),
                            ptz[:].rearrange("p a b -> p (a b)"),
                        )
                    else:
                        nc.vector.tensor_copy(
                            ztile[:].rearrange("p a b -> p (a b)"),
                            ptz[:].rearrange("p a b -> p (a b)"),
                        )
                    ztiles.append(ztile)
                return ssq, ztiles

            def batch_back(g, ssq, ztiles):
                t0, qn = g
                if probe == "dma":
                    getattr(nc, store_q).dma_start(
                        rowslice(out_dram, t0, qn),
                        og_c[:, :qn, :].rearrange("p a b -> p (a b)"),
                    )
                    return
                if probe != "pe":
                    zrec = stats_pool.tile([P, gq], F32, name="zrec")
                    nc.vector.reciprocal(zrec[:, :qn], ssq[:, :qn])
                    zscale = stats_pool.tile([P, gq], F32, name="zscale")
                    nc.scalar.activation(zscale[:, :qn], zrec[:, :qn], ACT.Sqrt)
                og = out_pool.tile([P, gq, D], BF16, name="ot")
                for qp in range(0, qn, fuse):
                    nb = min(fuse, qn - qp)
                    po2 = po_pool.tile([P, fuse, D], F32, name="psum_o")
                    for j in range(nb):
                        for k in range(KB):
                            nc.tensor.matmul(
                                po2[:, j, :],
                                ztiles[qp + j][:, k, :],
                                wT[:, k, :],
                                start=(k == 0),
                                stop=(k == KB - 1),
                            )
                    if probe == "pe":
                        nc.vector.tensor_copy(
                            og[:, qp : qp + nb, :].rearrange("p a b -> p (a b)"),
                            po2[:, :nb, :].rearrange("p a b -> p (a b)"),
                        )
                        continue
                    ab2 = abs_pool.tile([P, fuse, D], BF16, name="ab")
                    if scale_stage == "abs":
                        for j in range(nb):
                            q = qp + j
                            nc.scalar.activation(
                                ab2[:, j, :], po2[:, j, :], ACT.Abs,
                                scale=zscale[:, q : q + 1],
                            )
                    else:
                        nc.scalar.activation(
                            ab2[:, :nb, :].rearrange("p a b -> p (a b)"),
                            po2[:, :nb, :].rearrange("p a b -> p (a b)"),
                            ACT.Abs,
                        )
                    nc.vector.tensor_mul(
                        og[:, qp : qp + nb, :].rearrange("p a b -> p (a b)"),
                        po2[:, :nb, :].rearrange("p a b -> p (a b)"),
                        ab2[:, :nb, :].rearrange("p a b -> p (a b)"),
                    )
                    if scale_stage != "abs":
                        for j in range(nb):
                            q = qp + j
                            nc.vector.tensor_scalar_mul(
                                og[:, q, :], og[:, q, :], zscale[:, q : q + 1]
                            )
                if probe != "compute":
                    getattr(nc, store_q).dma_start(
                        rowslice(out_dram, t0, qn),
                        og[:, :qn, :].rearrange("p a b -> p (a b)"),
                    )

            def w_prep_stats():
                w_nat = wprep.tile([P, KB, D], F32)
                nc.sync.dma_start(
                    w_nat[:], w_dram[:].rearrange("(b p) d -> p b d", p=P)
                )
                wsq_scratch = wprep.tile([P, D], F32)
                wssq = wprep.tile([P, KB], F32)
                for b in range(KB):
                    nc.scalar.activation(
                        wsq_scratch[:], w_nat[:, b, :], ACT.Square,
                        accum_out=wssq[:, b : b + 1],
                    )
                wnrm = wprep.tile([P, KB], F32)
                nc.scalar.activation(wnrm[:], wssq[:], ACT.Sqrt)  # ||w||
                wnrm2 = wprep.tile([P, KB], F32)
                nc.scalar.activation(wnrm2[:], wnrm[:], ACT.Sqrt)  # ||w||^(1/2)
                wscale = wprep.tile([P, KB], F32)
                nc.vector.reciprocal(wscale[:], wnrm2[:])  # ||w||^(-1/2)
                w_nat2 = wprep.tile([P, KB, D], F32)
                nc.vector.tensor_copy(
                    w_nat2[:].rearrange("p a b -> p (a b)"),
                    w_nat[:].rearrange("p a b -> p (a b)"),
                )
                dsw = wprep.tile([P, KB, P], F32)
                for b in range(KB):
                    nc.vector.tensor_scalar_mul(
                        dsw[:, b, :], ident[:], wscale[:, b : b + 1]
                    )
                return w_nat2, dsw

            def w_prep_pe(w_nat2, dsw):
                for k in range(KB):
                    pw = pt_pool.tile([P, KB, P], F32, name="psum_t")
                    for b in range(KB):
                        nc.tensor.matmul(
                            pw[:, b, :],
                            w_nat2[:, b, k * P : (k + 1) * P],
                            dsw[:, b, :],
                        )
                    nc.vector.tensor_copy(  # f32 PSUM -> bf16 SBUF cast
                        wT[:, k, :], pw[:].rearrange("p a b -> p (a b)")
                    )

            def emit_passes(n_passes):
                all_groups = groups * n_passes
                fronts = {}
                for i in range(min(lookahead, len(all_groups))):
                    fronts[i] = batch_front(all_groups[i])
                yield
                for i in range(len(all_groups)):
                    if emit_order == "front_first":
                        if i + lookahead < len(all_groups):
                            fronts[i + lookahead] = batch_front(
                                all_groups[i + lookahead]
                            )
                        ssq, ztiles = fronts.pop(i)
                        batch_back(all_groups[i], ssq, ztiles)
                    else:
                        ssq, ztiles = fronts.pop(i)
                        batch_back(all_groups[i], ssq, ztiles)
                        if i + lookahead < len(all_groups):
                            fronts[i + lookahead] = batch_front(
                                all_groups[i + lookahead]
                            )

            w_nat2, dsw = w_prep_stats()
            if hw_loop:
                w_prep_pe(w_nat2, dsw)
                with tc.For_i(
                    0, hw_loop, 1,
                    hint_engines=(mybir.EngineType.PE, mybir.EngineType.Activation,
                                  mybir.EngineType.DVE, mybir.EngineType.SP,
                                  mybir.EngineType.Pool),
                ):
                    for _ in emit_passes(repeat):
                        pass
            else:
                gen = emit_passes(repeat)
                next(gen)
                w_prep_pe(w_nat2, dsw)
                for _ in gen:
                    pass

    nc.compile()
    return nc


def build_bench_kernel(rows, repeat=1, alias_rows=0, hw_loop=0, io="bf16", **kw):
    """Dispatcher for bench_slope: io="bf16" (new) or "f32" (v1 kernel)."""
    if io == "bf16" and "parts" not in kw:
        return build_kernel_bf16(rows, repeat, alias_rows, hw_loop, **kw)
    return build_kernel(rows, repeat, alias_rows, hw_loop, **kw)


_NC_CACHE: dict = {}


def z_np_dtype():
    """numpy dtype for the z DRAM tensor (ml_dtypes bfloat16)."""
    return mybir.dt.np(BF16)


def _get_nc(rows: int) -> bass.Bass:
    if rows not in _NC_CACHE:
        _NC_CACHE[rows] = build_kernel_bf16(rows)
    return _NC_CACHE[rows]


def kernel(z: np.ndarray, weight: np.ndarray) -> np.ndarray:
    """Full-input entry point: z [100000, 512] f32, weight [512, 512] f32.

    Device program is bf16-I/O (validated global rel err ~3e-3 vs the 2e-2
    gate): z is cast to bf16 on the host, the out tensor comes back bf16
    and is converted to f32 here. HBM traffic per core drops 51.4->25.7MB.
    """
    from concourse.bass_utils import run_bass_kernel_spmd

    z = np.ascontiguousarray(z, dtype=np.float32)
    weight = np.ascontiguousarray(weight, dtype=np.float32)
    n_rows = z.shape[0]
    per_core = -(-n_rows // N_CORES)
    per_core_pad = -(-per_core // P) * P

    nc = _get_nc(per_core_pad)

    bf = z_np_dtype()
    z16 = z.astype(bf)
    in_maps = []
    for c in range(N_CORES):
        lo = c * per_core
        hi = min(n_rows, (c + 1) * per_core)
        shard = np.zeros((per_core_pad, D), dtype=bf)
        shard[: hi - lo] = z16[lo:hi]
        in_maps.append({"z": shard, "w": weight})

    res = run_bass_kernel_spmd(nc, in_maps, core_ids=list(range(N_CORES)))
    out = np.empty((n_rows, D), dtype=np.float32)
    for c in range(N_CORES):
        lo = c * per_core
        hi = min(n_rows, (c + 1) * per_core)
        out[lo:hi] = res.results[c]["out"][: hi - lo].astype(np.float32)
    return out

